# revision 17
# baseline (speedup 1.0000x reference)
"""Circulant matmul kernel for Trainium2 (8 NeuronCores, SPMD).

Problem: out = input @ K + bias, K[i, k] = weight[(i - k) mod 4096],
input [1024, 4096] f32, weight/bias [4096] f32.

Algorithm — 2-level real CRT splitting of the circulant (exact):
  out = x (circ-conv) v + bias, v[j] = w[(-j) mod n].
  R[X]/(X^4096-1) ~ R[X]/(X^2048-1) x R[X]/(X^2048+1); the cyclic factor
  splits once more.  Device work becomes three dense GEMMs
    B0: y0 = x_m  @ M0   (nega-circulant 2048, M0 = 1/2 * NC(v_m))
    B1: y1 = x_pm @ M1   (nega-circulant 1024, M1 = 1/4 * NC(v_pm))
    B2: y2 = x_pp @ M2   (circulant      1024, M2 = 1/4 *  C(v_pp))
  6.29M weight elems instead of 16.8M (37%); split butterflies and the
  final recombine are host-side pure adds (exact).

Sharding: 8 cores = 4 column shards x 2 batch halves.  B0 is 4-way
column-sharded (512 cols/core); B1/B2 are 2-way sharded with even q
cores taking B1 and odd q cores taking B2 (512 cols each, identical
shapes on every core -> SPMD).  Each core also only needs ONE of
x_pm/x_pp, cutting X traffic 25%.  No collectives; host recombines.

Precision (all blocks share scale product sx*sw = 1024; epilogue
multiplies by 2^-10):
  B0 chunks 0-5   fp8e4m3 DoubleRow pairs (sx=4,  sw=256)
  B0 chunks 6-13  fp8e3m4 (4-bit mantissa; bf16-speed matmuls but half
                  the DMA bytes; sx=2, sw=512)
  B0 chunks 14-15 and all of B1/B2: bf16 (sx=4, sw=256)
  Sim rel err 1.82e-2 vs the 2e-2 gate (HW runs ~+0.07e-2 over sim).

Device schedule (per core):
  - X units on the sync HWDGE ring, W units on the scalar ring; every
    unit is one [128, 2, 512] tile = 1 DMA; issue order = consumption
    order.  6 PE warm-up matmuls on a dep-free scratch tile lift the
    HAM clock gate exactly until the first real unit lands.
  - Phase 1 co-major (unit at a time across all 4 batch tiles); phase 2
    finishes batch tiles in turn so ACT/DVE epilogues and the output
    DMAs (sync ring) overlap remaining matmuls.
  - PSUM: psA[bt] (B0) + psB[bt] (B1|B2) = 8 banks.  start=True clears
    has_written for a whole bank, so only the first matmul into each
    bank sets it.
"""

import numpy as np
import ml_dtypes

import concourse.bass as bass
import concourse.mybir as mybir
import concourse.tile as tile
from concourse import bacc
from concourse.bass import ts
from concourse.bass_utils import run_bass_kernel_spmd

N = 4096
BATCH = 1024
NCORES = 8
CQ = 4                    # B0 column shards
BH = 2                    # batch halves
R = BATCH // BH           # 512 rows per core
P = 128
BT = R // P               # 4 batch tiles per core

N0 = 2048                 # B0 block size; B1/B2 are 1024
KC = 512                  # output cols per core per block shard
NOUT = 2 * KC             # 1024 out cols per core (B0 shard | Bx shard)

# B0 chunk mix: 6 e4m3 (3 DR pairs, one 6-slot unit), 8 e3m4 (two 4-slot
# units), 2 bf16 (one 2-slot unit); Bx: 8 bf16 (two 4-slot units).
S8, S3, SB, SS = 6, 2, 2, 4       # slots per unit
NU8, NU3, NUB, NUS = 1, 4, 1, 2   # units per kind

SX8, SW8 = 4.0, 256.0
SX3, SW3 = 2.0, 512.0
SXB, SWB = 4.0, 256.0
INV_S = 2.0 ** -10
N_WARMUP = 6

BF16 = mybir.dt.bfloat16
FP8E4 = mybir.dt.float8e4
FP8E3 = mybir.dt.float8e3
F32 = mybir.dt.float32


def build_nc():
    nc = bacc.Bacc("TRN2", target_bir_lowering=False, debug=False)

    def din(name, nu, slots, dt):
        return nc.dram_tensor(name, [nu * P, slots, KC], dt, kind="ExternalInput").ap() \
                 .rearrange("(u ci) s k -> ci u s k", ci=P)

    x8_r = din("x8", NU8, S8, FP8E4)
    x3_r = din("x3", NU3, S3, FP8E3)
    xb_r = din("xb", NUB, SB, BF16)
    xs_r = din("xs", NUS, SS, BF16)
    w8_r = din("w8", NU8, S8, FP8E4)
    w3_r = din("w3", NU3, S3, FP8E3)
    wb_r = din("wb", NUB, SB, BF16)
    ws_r = din("ws", NUS, SS, BF16)
    out_d = nc.dram_tensor("out", [R, NOUT], BF16, kind="ExternalOutput").ap()

    with tile.TileContext(nc) as tc:
        with (
            tc.tile_pool(name="p2", bufs=1) as p2,
            tc.tile_pool(name="p4", bufs=2) as p4,
            tc.tile_pool(name="cpool", bufs=1) as cpool,
            tc.tile_pool(name="opool", bufs=4) as opool,
            tc.tile_pool(name="psumA", bufs=BT, space="PSUM") as psumA_pool,
            tc.tile_pool(name="psumB", bufs=BT, space="PSUM") as psumB_pool,
        ):
            scratch = cpool.tile([P, P + KC], BF16, tag="scratch")
            nc.gpsimd.memset(scratch[:, 0:1], 0.125)

            # kinds: A=B0 bf16, E=B0 e3m4, F=B0 e4m3 DR pairs, S=Bx bf16.
            # Each unit is one [128, 4, 512] tile = 4 chunks (F: 2 DR pairs).
            # E0 leads: half-size bytes, so the PE starts right as the
            # warm-ups finish.  The last slots of A0/S1 close the psum
            # banks bt-major in phase 2.
            cfg = {"A": (p2, xb_r, wb_r, BF16, SB, "xb", "wb"),
                   "E": (p4, x3_r, w3_r, FP8E3, S3, "x3", "w3"),
                   "F": (p2, x8_r, w8_r, FP8E4, S8, "x8", "w8"),
                   "S": (p4, xs_r, ws_r, BF16, SS, "xs", "ws")}
            # (kind, unit, slot_lo, slot_hi).  All fp8 B0 units lead:
            # highest PE-work-per-DMA-byte, so the PE builds a work buffer
            # before the big bf16 S transfers contend for HBM.
            ph1 = [("E", 0, 0, 2), ("E", 1, 0, 2), ("E", 2, 0, 2),
                   ("E", 3, 0, 2), ("F", 0, 0, 6), ("S", 0, 0, 4),
                   ("S", 1, 0, 2)]
            ph2 = [("A", 0, 0, 2), ("S", 1, 2, 4)]
            dma_order = [("E", 0), ("E", 1), ("E", 2), ("E", 3),
                         ("F", 0), ("S", 0), ("S", 1), ("A", 0)]

            xt, wt = {}, {}
            for kind, u in dma_order:
                pool, xr, wr, dt, slots, xtag, wtag = cfg[kind]
                xtt = pool.tile([P, slots, R], dt, tag=xtag)
                nc.sync.dma_start(xtt[:], xr[:, u, :, :])
                xt[(kind, u)] = xtt
                wtt = pool.tile([P, slots, KC], dt, tag=wtag)
                nc.scalar.dma_start(wtt[:], wr[:, u, :, :])
                wt[(kind, u)] = wtt

            psA = [psumA_pool.tile([P, KC], F32, tag="psA", name=f"psA{b}")
                   for b in range(BT)]
            psB = [psumB_pool.tile([P, KC], F32, tag="psB", name=f"psB{b}")
                   for b in range(BT)]

            for i in range(N_WARMUP):
                nc.tensor.matmul(
                    psA[i % BT][:],
                    scratch[:, P:2 * P],
                    scratch[:, P:P + KC],
                    start=True, stop=True,
                )

            started = set()

            def unit_mms(kind, u, bt, s_lo, s_hi, stop=False):
                ps = psA[bt] if kind in ("A", "E", "F") else psB[bt]
                if kind == "F":
                    for pp in range(s_lo // 2, s_hi // 2):
                        nc.tensor.matmul(
                            ps[:],
                            xt[(kind, u)][:, 2 * pp:2 * pp + 2, ts(bt, P)],
                            wt[(kind, u)][:, 2 * pp:2 * pp + 2, :],
                            start=False, stop=False,
                            perf_mode=mybir.MatmulPerfMode.DoubleRow,
                        )
                    return
                bank = "A" if kind in ("A", "E") else "B"
                for s in range(s_lo, s_hi):
                    key = (bank, bt)
                    st = key not in started
                    started.add(key)
                    nc.tensor.matmul(
                        ps[:],
                        xt[(kind, u)][:, s, ts(bt, P)],
                        wt[(kind, u)][:, s, :],
                        start=st, stop=(stop and s == s_hi - 1),
                    )

            for kind, u, lo, hi in ph1:
                for bt in range(BT):
                    unit_mms(kind, u, bt, lo, hi)

            for bt in range(BT):
                for kind, u, lo, hi in ph2:
                    unit_mms(kind, u, bt, lo, hi, stop=True)
                out_sb = opool.tile([P, NOUT], BF16, tag="osb")
                nc.scalar.activation(
                    out_sb[:, 0:KC], psA[bt][:],
                    mybir.ActivationFunctionType.Copy, scale=INV_S,
                )
                nc.vector.tensor_scalar_mul(
                    out_sb[:, KC:NOUT], psB[bt][:], INV_S,
                )
                ring = nc.sync if bt % 2 == 0 else nc.scalar
                ring.dma_start(out_d[ts(bt, P), :], out_sb[:])

    nc.compile()
    return nc


def _nega_circ(v, m):
    i = np.arange(m)
    d = i[None, :] - i[:, None]
    return v[d % m] * np.where(d < 0, -1.0, 1.0)


def _circ(v, m):
    i = np.arange(m)
    return v[(i[None, :] - i[:, None]) % m]


def _pack(a, slots, dt, clip=False):
    """[(nu*slots)*P, k] chunk-major f32 -> [(nu*P), slots, k] in dtype dt."""
    if clip:
        a = np.clip(a, -15.5, 15.5)
    k = a.shape[1]
    nu = a.shape[0] // (slots * P)
    return np.ascontiguousarray(
        a.reshape(nu, slots, P, k).transpose(0, 2, 1, 3).reshape(nu * P, slots, k)
    ).astype(dt)


def prepare_in_maps(input, weight, bias):
    x = np.asarray(input, dtype=np.float64)
    w = np.asarray(weight, dtype=np.float64)

    v = w[(-np.arange(N)) % N]
    xp = x[:, :2048] + x[:, 2048:]
    xm = x[:, :2048] - x[:, 2048:]
    vp = v[:2048] + v[2048:]
    vm = v[:2048] - v[2048:]
    xpp = xp[:, :1024] + xp[:, 1024:]
    xpm = xp[:, :1024] - xp[:, 1024:]
    vpp = vp[:1024] + vp[1024:]
    vpm = vp[:1024] - vp[1024:]

    M0 = _nega_circ(vm, N0) * 0.5
    M1 = _nega_circ(vpm, 1024) * 0.25
    M2 = _circ(vpp, 1024) * 0.25

    BF = ml_dtypes.bfloat16
    E4 = ml_dtypes.float8_e4m3fn
    E3 = ml_dtypes.float8_e3m4

    xmT = np.ascontiguousarray(xm.T).astype(np.float32)     # [2048, 1024]
    xpmT = np.ascontiguousarray(xpm.T).astype(np.float32)   # [1024, 1024]
    xppT = np.ascontiguousarray(xpp.T).astype(np.float32)
    M0 = M0.astype(np.float32)
    M1 = M1.astype(np.float32)
    M2 = M2.astype(np.float32)

    in_maps = []
    for h in range(BH):
        rs = slice(h * R, (h + 1) * R)
        x8 = _pack(xmT[0:768, rs] * SX8, S8, E4)
        x3 = _pack(xmT[768:1792, rs] * SX3, S3, E3, clip=True)
        xb = _pack(xmT[1792:2048, rs] * SXB, SB, BF)
        xs_b1 = _pack(xpmT[:, rs] * SXB, SS, BF)
        xs_b2 = _pack(xppT[:, rs] * SXB, SS, BF)
        for q in range(CQ):
            cs = slice(q * KC, (q + 1) * KC)
            Mx = M1 if q % 2 == 0 else M2
            hs = slice((q // 2) * KC, (q // 2 + 1) * KC)
            in_maps.append({
                "x8": x8, "x3": x3, "xb": xb,
                "xs": xs_b1 if q % 2 == 0 else xs_b2,
                "w8": _pack(M0[0:768, cs] * SW8, S8, E4),
                "w3": _pack(M0[768:1792, cs] * SW3, S3, E3, clip=True),
                "wb": _pack(M0[1792:2048, cs] * SWB, SB, BF),
                "ws": _pack(Mx[:, hs] * SWB, SS, BF),
            })
    # core order: core = h*CQ + q
    return in_maps


def assemble(results, bias):
    """results: per-core {"out": [R, NOUT] bf16}; host butterflies + bias."""
    y0 = np.empty((BATCH, N0), np.float32)
    y1 = np.empty((BATCH, 1024), np.float32)
    y2 = np.empty((BATCH, 1024), np.float32)
    for h in range(BH):
        rs = slice(h * R, (h + 1) * R)
        for q in range(CQ):
            o = results[h * CQ + q]["out"].astype(np.float32)
            y0[rs, q * KC:(q + 1) * KC] = o[:, 0:KC]
            dst = y1 if q % 2 == 0 else y2
            dst[rs, (q // 2) * KC:(q // 2 + 1) * KC] = o[:, KC:NOUT]
    yp = np.concatenate([y2 + y1, y2 - y1], axis=1)
    out = np.concatenate([yp + y0, yp - y0], axis=1)
    return out + np.asarray(bias, np.float32)[None, :]


_NC_CACHE = None


def _get_nc():
    global _NC_CACHE
    if _NC_CACHE is None:
        _NC_CACHE = build_nc()
    return _NC_CACHE


def kernel(**inputs):
    nc = _get_nc()
    in_maps = prepare_in_maps(inputs["input"], inputs["weight"], inputs["bias"])
    res = run_bass_kernel_spmd(nc, in_maps, list(range(NCORES)))
    return assemble(res.results, inputs["bias"])


# revision 18
# speedup vs baseline: 1.1986x; 1.1986x over previous
"""Circulant matmul kernel for Trainium2 (8 NeuronCores, SPMD).

Problem: out = input @ K + bias, K[i, k] = weight[(i - k) mod 4096],
input [1024, 4096] f32, weight/bias [4096] f32.

Algorithm — 2-level real CRT splitting of the circulant (exact):
  out = x (circ-conv) v + bias, v[j] = w[(-j) mod n].
  R[X]/(X^4096-1) ~ R[X]/(X^2048-1) x R[X]/(X^2048+1); the cyclic factor
  splits once more.  Device work becomes three dense GEMMs
    B0: y0 = x_m  @ M0   (nega-circulant 2048, M0 = 1/2 * NC(v_m))
    B1: y1 = x_pm @ M1   (nega-circulant 1024, M1 = 1/4 * NC(v_pm))
    B2: y2 = x_pp @ M2   (circulant      1024, M2 = 1/4 *  C(v_pp))
  6.29M weight elems instead of 16.8M (37%); split butterflies and the
  final recombine are host-side pure adds (exact).

Sharding: 8 cores = 4 column shards x 2 batch halves.  B0 is 4-way
column-sharded (512 cols/core); B1/B2 are 2-way sharded with even q
cores taking B1 and odd q cores taking B2 (512 cols each, identical
shapes on every core -> SPMD).  Each core also only needs ONE of
x_pm/x_pp, cutting X traffic 25%.  No collectives; host recombines.

Precision (all blocks share scale product sx*sw = 1024; epilogue
multiplies by 2^-10):
  B0 chunks 0-5   fp8e4m3 DoubleRow pairs (sx=4,  sw=256)
  B0 chunks 6-13  fp8e3m4 (4-bit mantissa; bf16-speed matmuls but half
                  the DMA bytes; sx=2, sw=512)
  B0 chunks 14-15 and all of B1/B2: bf16 (sx=4, sw=256)
  Sim rel err 1.82e-2 vs the 2e-2 gate (HW runs ~+0.07e-2 over sim).

Device schedule (per core):
  - X units on the sync HWDGE ring, W units on the scalar ring; every
    unit is one [128, 2, 512] tile = 1 DMA; issue order = consumption
    order.  6 PE warm-up matmuls on a dep-free scratch tile lift the
    HAM clock gate exactly until the first real unit lands.
  - Phase 1 co-major (unit at a time across all 4 batch tiles); phase 2
    finishes batch tiles in turn so ACT/DVE epilogues and the output
    DMAs (sync ring) overlap remaining matmuls.
  - PSUM: psA[bt] (B0) + psB[bt] (B1|B2) = 8 banks.  start=True clears
    has_written for a whole bank, so only the first matmul into each
    bank sets it.
"""

import numpy as np
import ml_dtypes

import concourse.bass as bass
import concourse.mybir as mybir
import concourse.tile as tile
from concourse import bacc
from concourse.bass import ts
from concourse.bass_utils import run_bass_kernel_spmd

N = 4096
BATCH = 1024
NCORES = 8
CQ = 4                    # B0 column shards
BH = 2                    # batch halves
R = BATCH // BH           # 512 rows per core
P = 128
BT = R // P               # 4 batch tiles per core

N0 = 2048                 # B0 block size; B1/B2 are 1024
KC = 512                  # output cols per core per block shard
NOUT = 2 * KC             # 1024 out cols per core (B0 shard | Bx shard)

# B0 chunk mix: 6 e4m3 (3 DR pairs, one 6-slot unit), 8 e3m4 (two 4-slot
# units), 2 bf16 (one 2-slot unit); Bx: 8 bf16 (two 4-slot units).
S8, S3, SB, SS = 6, 2, 2, 4       # slots per unit
NU8, NU3, NUB, NUS = 1, 4, 1, 2   # units per kind

SX8, SW8 = 4.0, 256.0
SX3, SW3 = 2.0, 512.0
SXB, SWB = 4.0, 256.0
INV_S = 2.0 ** -10
N_WARMUP = 6

BF16 = mybir.dt.bfloat16
FP8E4 = mybir.dt.float8e4
FP8E3 = mybir.dt.float8e3
F32 = mybir.dt.float32


def build_nc():
    nc = bacc.Bacc("TRN2", target_bir_lowering=False, debug=False)

    def din(name, nu, slots, dt):
        return nc.dram_tensor(name, [nu * P, slots, KC], dt, kind="ExternalInput").ap() \
                 .rearrange("(u ci) s k -> ci u s k", ci=P)

    x8_r = din("x8", NU8, S8, FP8E4)
    x3_r = din("x3", NU3, S3, FP8E3)
    xb_r = din("xb", NUB, SB, BF16)
    xs_r = din("xs", NUS, SS, BF16)
    w8_r = din("w8", NU8, S8, FP8E4)
    w3_r = din("w3", NU3, S3, FP8E3)
    wb_r = din("wb", NUB, SB, BF16)
    ws_r = din("ws", NUS, SS, BF16)
    out_d = nc.dram_tensor("out", [R, NOUT], BF16, kind="ExternalOutput").ap()

    with tile.TileContext(nc) as tc:
        with (
            tc.tile_pool(name="p2", bufs=1) as p2,
            tc.tile_pool(name="p4", bufs=4) as p4,
            tc.tile_pool(name="cpool", bufs=1) as cpool,
            tc.tile_pool(name="opool", bufs=4) as opool,
            tc.tile_pool(name="psumA", bufs=BT, space="PSUM") as psumA_pool,
            tc.tile_pool(name="psumB", bufs=BT, space="PSUM") as psumB_pool,
        ):
            scratch = cpool.tile([P, P + KC], BF16, tag="scratch")
            nc.gpsimd.memset(scratch[:, 0:1], 0.125)

            # kinds: A=B0 bf16, E=B0 e3m4, F=B0 e4m3 DR pairs, S=Bx bf16.
            # Each unit is one [128, 4, 512] tile = 4 chunks (F: 2 DR pairs).
            # E0 leads: half-size bytes, so the PE starts right as the
            # warm-ups finish.  The last slots of A0/S1 close the psum
            # banks bt-major in phase 2.
            cfg = {"A": (p2, xb_r, wb_r, BF16, SB, "xb", "wb"),
                   "E": (p4, x3_r, w3_r, FP8E3, S3, "x3", "w3"),
                   "F": (p2, x8_r, w8_r, FP8E4, S8, "x8", "w8"),
                   "S": (p4, xs_r, ws_r, BF16, SS, "xs", "ws")}
            # (kind, unit, slot_lo, slot_hi).  All fp8 B0 units lead:
            # highest PE-work-per-DMA-byte, so the PE builds a work buffer
            # before the big bf16 S transfers contend for HBM.
            ph1 = [("E", 0, 0, 2), ("E", 1, 0, 2), ("E", 2, 0, 2),
                   ("E", 3, 0, 2), ("F", 0, 0, 6), ("S", 0, 0, 4),
                   ("S", 1, 0, 2)]
            ph2 = [("A", 0, 0, 2), ("S", 1, 2, 4)]
            dma_order = [("E", 0), ("E", 1), ("E", 2), ("E", 3),
                         ("F", 0), ("S", 0), ("S", 1), ("A", 0)]

            xt, wt = {}, {}
            for kind, u in dma_order:
                pool, xr, wr, dt, slots, xtag, wtag = cfg[kind]
                xtt = pool.tile([P, slots, R], dt, tag=xtag)
                nc.sync.dma_start(xtt[:], xr[:, u, :, :])
                xt[(kind, u)] = xtt
                wtt = pool.tile([P, slots, KC], dt, tag=wtag)
                nc.scalar.dma_start(wtt[:], wr[:, u, :, :])
                wt[(kind, u)] = wtt

            psA = [psumA_pool.tile([P, KC], F32, tag="psA", name=f"psA{b}")
                   for b in range(BT)]
            psB = [psumB_pool.tile([P, KC], F32, tag="psB", name=f"psB{b}")
                   for b in range(BT)]

            for i in range(N_WARMUP):
                nc.tensor.matmul(
                    psA[i % BT][:],
                    scratch[:, P:2 * P],
                    scratch[:, P:P + KC],
                    start=True, stop=True,
                )

            started = set()

            def unit_mms(kind, u, bt, s_lo, s_hi, stop=False):
                ps = psA[bt] if kind in ("A", "E", "F") else psB[bt]
                if kind == "F":
                    for pp in range(s_lo // 2, s_hi // 2):
                        nc.tensor.matmul(
                            ps[:],
                            xt[(kind, u)][:, 2 * pp:2 * pp + 2, ts(bt, P)],
                            wt[(kind, u)][:, 2 * pp:2 * pp + 2, :],
                            start=False, stop=False,
                            perf_mode=mybir.MatmulPerfMode.DoubleRow,
                        )
                    return
                bank = "A" if kind in ("A", "E") else "B"
                for s in range(s_lo, s_hi):
                    key = (bank, bt)
                    st = key not in started
                    started.add(key)
                    nc.tensor.matmul(
                        ps[:],
                        xt[(kind, u)][:, s, ts(bt, P)],
                        wt[(kind, u)][:, s, :],
                        start=st, stop=(stop and s == s_hi - 1),
                    )

            for kind, u, lo, hi in ph1:
                for bt in range(BT):
                    unit_mms(kind, u, bt, lo, hi)

            for bt in range(BT):
                for kind, u, lo, hi in ph2:
                    unit_mms(kind, u, bt, lo, hi, stop=True)
                out_sb = opool.tile([P, NOUT], BF16, tag="osb")
                nc.scalar.activation(
                    out_sb[:, 0:KC], psA[bt][:],
                    mybir.ActivationFunctionType.Copy, scale=INV_S,
                )
                nc.vector.tensor_scalar_mul(
                    out_sb[:, KC:NOUT], psB[bt][:], INV_S,
                )
                ring = nc.sync if bt % 2 == 0 else nc.scalar
                ring.dma_start(out_d[ts(bt, P), :], out_sb[:])

    nc.compile()
    return nc


def _nega_circ(v, m):
    i = np.arange(m)
    d = i[None, :] - i[:, None]
    return v[d % m] * np.where(d < 0, -1.0, 1.0)


def _circ(v, m):
    i = np.arange(m)
    return v[(i[None, :] - i[:, None]) % m]


def _pack(a, slots, dt, clip=False):
    """[(nu*slots)*P, k] chunk-major f32 -> [(nu*P), slots, k] in dtype dt."""
    if clip:
        a = np.clip(a, -15.5, 15.5)
    k = a.shape[1]
    nu = a.shape[0] // (slots * P)
    return np.ascontiguousarray(
        a.reshape(nu, slots, P, k).transpose(0, 2, 1, 3).reshape(nu * P, slots, k)
    ).astype(dt)


def prepare_in_maps(input, weight, bias):
    x = np.asarray(input, dtype=np.float64)
    w = np.asarray(weight, dtype=np.float64)

    v = w[(-np.arange(N)) % N]
    xp = x[:, :2048] + x[:, 2048:]
    xm = x[:, :2048] - x[:, 2048:]
    vp = v[:2048] + v[2048:]
    vm = v[:2048] - v[2048:]
    xpp = xp[:, :1024] + xp[:, 1024:]
    xpm = xp[:, :1024] - xp[:, 1024:]
    vpp = vp[:1024] + vp[1024:]
    vpm = vp[:1024] - vp[1024:]

    M0 = _nega_circ(vm, N0) * 0.5
    M1 = _nega_circ(vpm, 1024) * 0.25
    M2 = _circ(vpp, 1024) * 0.25

    BF = ml_dtypes.bfloat16
    E4 = ml_dtypes.float8_e4m3fn
    E3 = ml_dtypes.float8_e3m4

    xmT = np.ascontiguousarray(xm.T).astype(np.float32)     # [2048, 1024]
    xpmT = np.ascontiguousarray(xpm.T).astype(np.float32)   # [1024, 1024]
    xppT = np.ascontiguousarray(xpp.T).astype(np.float32)
    M0 = M0.astype(np.float32)
    M1 = M1.astype(np.float32)
    M2 = M2.astype(np.float32)

    in_maps = []
    for h in range(BH):
        rs = slice(h * R, (h + 1) * R)
        x8 = _pack(xmT[0:768, rs] * SX8, S8, E4)
        x3 = _pack(xmT[768:1792, rs] * SX3, S3, E3, clip=True)
        xb = _pack(xmT[1792:2048, rs] * SXB, SB, BF)
        xs_b1 = _pack(xpmT[:, rs] * SXB, SS, BF)
        xs_b2 = _pack(xppT[:, rs] * SXB, SS, BF)
        for q in range(CQ):
            cs = slice(q * KC, (q + 1) * KC)
            Mx = M1 if q % 2 == 0 else M2
            hs = slice((q // 2) * KC, (q // 2 + 1) * KC)
            in_maps.append({
                "x8": x8, "x3": x3, "xb": xb,
                "xs": xs_b1 if q % 2 == 0 else xs_b2,
                "w8": _pack(M0[0:768, cs] * SW8, S8, E4),
                "w3": _pack(M0[768:1792, cs] * SW3, S3, E3, clip=True),
                "wb": _pack(M0[1792:2048, cs] * SWB, SB, BF),
                "ws": _pack(Mx[:, hs] * SWB, SS, BF),
            })
    # core order: core = h*CQ + q
    return in_maps


def assemble(results, bias):
    """results: per-core {"out": [R, NOUT] bf16}; host butterflies + bias."""
    y0 = np.empty((BATCH, N0), np.float32)
    y1 = np.empty((BATCH, 1024), np.float32)
    y2 = np.empty((BATCH, 1024), np.float32)
    for h in range(BH):
        rs = slice(h * R, (h + 1) * R)
        for q in range(CQ):
            o = results[h * CQ + q]["out"].astype(np.float32)
            y0[rs, q * KC:(q + 1) * KC] = o[:, 0:KC]
            dst = y1 if q % 2 == 0 else y2
            dst[rs, (q // 2) * KC:(q // 2 + 1) * KC] = o[:, KC:NOUT]
    yp = np.concatenate([y2 + y1, y2 - y1], axis=1)
    out = np.concatenate([yp + y0, yp - y0], axis=1)
    return out + np.asarray(bias, np.float32)[None, :]


_NC_CACHE = None


def _get_nc():
    global _NC_CACHE
    if _NC_CACHE is None:
        _NC_CACHE = build_nc()
    return _NC_CACHE


def kernel(**inputs):
    nc = _get_nc()
    in_maps = prepare_in_maps(inputs["input"], inputs["weight"], inputs["bias"])
    res = run_bass_kernel_spmd(nc, in_maps, list(range(NCORES)))
    return assemble(res.results, inputs["bias"])
